# revision 2
# baseline (speedup 1.0000x reference)
"""DynamicConv Trainium2 kernel.

Math (B=1, L=2048, D=128, E=128, F=8, K1=K2=3, M=K2*D=384):
  f   = u @ proj                                   [L, F]
  kp[l,e,m] = sum_{k1,fc} f_pad[l+k1-1,fc] * W[e,k1,fc,m] + b[e,m]
  out[l,e]  = sum_{d,k2} u_pad[l+k2-1,d] * kp[l,e,d*K2+k2]

Factorized as out[l,e] = sum_j f_tap[l,j] * A_j[l,e] + bias_t[l,e] with
A/bias/f all produced by 3 shifted bf16 matmuls per l-tile of 128 positions
accumulated in PSUM; proj columns are embedded in the rhs so f_tap falls out
of the same matmuls.  PSUM layout (26-wide blocks):
  e*26 + j  (j<24): A_j[l,e];  j=24: bias_t[l,e];  j=25: zero
  416 + j   (j<24): f_tap[l,j]
The combine runs on a CUSTOM DVE op (registered at build time through the
documented dve_ops extension point): SEG_MUL_CUMSUM computes
  cum[p, k] = sum_{k'<=k} in0[p, k'] * in1[p, k']
in one 1x pass, reading the A/bias columns STRAIGHT FROM PSUM in fp32
(no ACT bulk copy, no separate multiply).  Per-(e) segment sums are then
boundary differences of the global cumsum:
  out[l, e] = cum[l, e*26+25] - cum[l, e*26-1]     (cum[-1] := 0)
a [128, q, 16] strided tensor_tensor subtract.  in1 is "fone" =
f_tap replicated over the 16 e-blocks with 1.0 at j=24 (bias) and 0.0 at
j=25, materialized per group by ONE broadcast ACT copy straight from the
PSUM f block (GPSIMD does no streaming work at all - its SBUF port is
shared with the DVE and any concurrent GPSIMD traffic poisons DVE ops).
Per pair-group engine cost: ACT ~0.95us, DVE ~1.25us -- everything fits
under the ~10us matmul stream, and PSUM is freed by the scan itself.

The PE warm-up streams the framework's bf16 const tile via stride-0 APs
(no memset dependency) and is trimmed to 5 matmuls; the first u chunk is
split in half so the first tile's inputs land ~1.5us earlier.  Input DMAs:
sync(w0,u3), scalar(u0a,u0b,u1), gpsimd(w1,w2,u2).  Outputs are batched
8 l-tiles per DMA; the host un-permutes.

Measured ~27.0-27.4us per core on hardware (the empty-kernel launch
floor - preamble barriers + the walrus epilogue's full semaphore wipe -
is ~13us of that); rel err ~3.9e-3 vs the fp32 reference.

E is sharded 8 ways (16 channels/core); u is replicated.
"""

import numpy as np
import ml_dtypes

BF16 = ml_dtypes.bfloat16

B, L, D = 1, 2048, 128
E, F = 128, 8
K1, K2 = 3, 3
M = K2 * D
NCORES = 8
EL = E // NCORES          # 16 output channels per core
NJ = K1 * F               # 24 (k1, fc) pairs
NJB = NJ + 2              # 26-wide blocks: A(24) + bias + zero
NA = EL * NJB             # 416 A/bias columns
NW = NA + NJ              # 440 matmul columns (f block is 24 wide)
LT = 128                  # l-tile size
NT = L // LT              # 16 l-tiles
GT = 8                    # l-tiles per output DMA group
NG = NT // GT             # output groups
UC = 4                    # l-tiles per u chunk
UCOLS = UC * LT + 2       # 514
UH = 2 * LT + 2           # 258: first chunk is split for an earlier start
NU = NT // UC             # 4 u chunks
PSW = 512                 # psum columns per sub-tile (bank-aligned)
NWARM = 7                 # PE clock-ramp matmuls before the real stream
WARMC = 512               # warm-up matmul column count
QT = 2                    # max l-tiles per group
# pairs, with the last two tiles as singles: the drain chain after the
# final matmul is fone(0.55)+scan(0.56)+diff(0.1) instead of ~2.2us.
GROUPS = [[0, 1], [2, 3], [4, 5], [6, 7], [8, 9], [10, 11], [12, 13],
          [14], [15]]

_OP_NAME = "SEG_MUL_CUMSUM_ANT"


def _ensure_custom_op():
    """Register the fused multiply+cumsum DVE op via the documented
    dve_ops extension point (idempotent)."""
    import concourse.dve_ops as dve_ops

    for op in dve_ops.OPS:
        if op.name == _OP_NAME:
            return op
    from concourse.dve_spec import AluOp, Spec, Src0, Src1, lower, scan
    from concourse.dve_spec import _has_src1
    from concourse.dve_uop import DveOpSpec

    def _ref(in0, in1, s0, s1, imm2):
        p, rest = in0.shape[0], int(np.prod(in0.shape[1:]))
        prod = (in0.astype(np.float32) * in1.astype(np.float32)).reshape(p, rest)
        return np.cumsum(prod, axis=1).reshape(in0.shape)

    spec = Spec(body=scan(AluOp.ADD, Src0 * Src1), reference=_ref)
    row = 1 + len(dve_ops.OPS)
    assert row < 0x20, "custom-DVE row field overflow"
    shas = {}
    for ver in ("v3", "v4"):
        u = lower(spec, ver=ver)
        shas[ver] = DveOpSpec(
            name=_OP_NAME, opcode=row, uops=u, rd1_en=_has_src1(spec)
        ).sha(ver)
    op = dve_ops.DveOp(_OP_NAME, spec, subdim=False, uops_sha=shas)
    dve_ops.OPS.append(op)
    dve_ops.CUSTOM_DVE_SPECS[op.name] = op.spec
    dve_ops._SUB_OPCODE_FOR_NAME[op.name] = row
    return op


def _build_program():
    import concourse.bass as bass
    import concourse.bacc as bacc
    import concourse.tile as tile
    from concourse import mybir

    seg_op = _ensure_custom_op()

    f32 = mybir.dt.float32
    bf16 = mybir.dt.bfloat16
    nc = bacc.Bacc("TRN2", target_bir_lowering=False, debug=False)

    u_dram = nc.dram_tensor("u_padt", [D, L + 2], bf16, kind="ExternalInput")
    w_dram = nc.dram_tensor("w_aug", [D, K2 * NW], bf16, kind="ExternalInput")
    o_dram = nc.dram_tensor("out", [NG, D, GT * EL], f32, kind="ExternalOutput")
    # keep-alive sink for the PE warm-up matmuls (ignored by the host)
    warm_dram = nc.dram_tensor("warm", [1, 1], bf16, kind="ExternalOutput")

    with tile.TileContext(nc) as tc:
        import contextlib

        with contextlib.ExitStack() as ctx:
            const_pool = ctx.enter_context(tc.tile_pool(name="const", bufs=1))
            psum_pool = ctx.enter_context(
                tc.tile_pool(name="psum", bufs=4, space="PSUM")
            )
            fpool = ctx.enter_context(tc.tile_pool(name="ftile", bufs=4))
            fonep = ctx.enter_context(tc.tile_pool(name="fone", bufs=4))
            cump = ctx.enter_context(tc.tile_pool(name="cum", bufs=4))
            outp = ctx.enter_context(tc.tile_pool(name="outt", bufs=2))

            # u chunks: the first 4 tiles use two 2-tile chunks (earlier
            # start + precise DMA deps); the rest use 4-tile chunks.
            u_sbs = []
            for g in range(NU):
                u_g = const_pool.tile([D, UCOLS], bf16, tag=f"u{g}", name=f"u{g}")
                u_sbs.append(u_g)
            ua = const_pool.tile([D, UH], bf16, name="ua")
            ub = const_pool.tile([D, UH], bf16, name="ub")
            w_sb = const_pool.tile([D, K2 * NW], bf16)

            def u_window(t, k):
                # [128 x 128] lhs window for tile t, shift k
                if t < 2:
                    return ua[:, t * LT + k : t * LT + k + LT]
                if t < 4:
                    return ub[:, (t - 2) * LT + k : (t - 2) * LT + k + LT]
                return u_sbs[t // UC][:, (t % UC) * LT + k : (t % UC) * LT + k + LT]

            def dma_u(g, eng):
                eng.dma_start(
                    out=u_sbs[g][:],
                    in_=u_dram[:, g * UC * LT : g * UC * LT + UCOLS],
                )

            def dma_w(k, eng):
                eng.dma_start(
                    out=w_sb[:, k * NW : (k + 1) * NW],
                    in_=w_dram[:, k * NW : (k + 1) * NW],
                )

            # need-order: w0/ua/w1 gate tile 0; the first u chunk is split
            # so the stream can start ~1.5us earlier; sync frees up early
            # for the output DMAs.
            dma_w(0, nc.sync)
            nc.scalar.dma_start(out=ua[:], in_=u_dram[:, 0:UH])
            dma_w(1, nc.gpsimd)
            nc.scalar.dma_start(out=ub[:], in_=u_dram[:, 2 * LT : 2 * LT + UH])
            dma_w(2, nc.sync)
            dma_u(1, nc.scalar)
            dma_u(2, nc.gpsimd)
            dma_u(3, nc.sync)

            # PE warm-up on the framework's pre-initialized bf16 const tile
            # via stride-0 APs: no memset dependency, so the DVFS ramp starts
            # the moment the PE enters the body.
            one_ap = nc.const_aps.aps[(bf16, 1.0)]
            warm_in0 = bass.AP(
                tensor=one_ap.tensor, offset=one_ap.offset,
                ap=[one_ap.ap[0], [0, LT]],
            )
            warm_in1 = bass.AP(
                tensor=one_ap.tensor, offset=one_ap.offset,
                ap=[one_ap.ap[0], [0, WARMC]],
            )
            warm_ps = psum_pool.tile([LT, QT, PSW], f32, tag="ps", name="warm_ps")
            for i in range(NWARM):
                nc.tensor.matmul(
                    warm_ps[:, 0, 0:WARMC],
                    warm_in0,
                    warm_in1,
                    start=(i == 0),
                    stop=(i == NWARM - 1),
                )
            warm_1 = bass.AP(
                tensor=one_ap.tensor, offset=one_ap.offset,
                ap=[[one_ap.ap[0][0], 1], [1, 1]],
            )
            nc.sync.dma_start(out=warm_dram[:], in_=warm_1)

            # rotating buffers: fone gets its 1.0 (bias) / 0.0 (pad) block
            # cols once; cum gets its seed column (global cumsum "-1" = 0).
            fones, cums = [], []
            for b in range(4):
                fone = fonep.tile([LT, QT, NA], bf16, tag="fone", name=f"fone{b}")
                f4 = fone[:].rearrange("p q (e j) -> p q e j", j=NJB)
                nc.gpsimd.memset(f4[:, :, :, NJ : NJ + 1], 1.0)
                nc.gpsimd.memset(f4[:, :, :, NJ + 1 : NJB], 0.0)
                fones.append(fone)
                # width 1 + QT*NA + NJB: the trailing NJB cols are slack so
                # the boundary-view slices stay in bounds (never read).
                cum = cump.tile(
                    [LT, 1 + QT * NA + NJB], f32, tag="cum", name=f"cum{b}"
                )
                nc.gpsimd.memset(cum[:, 0:1], 0.0)
                cums.append(cum)


            o_bigs = {}

            for g, tiles in enumerate(GROUPS):
                q = len(tiles)
                if tiles[0] % GT == 0:
                    o_bigs[tiles[0] // GT] = outp.tile(
                        [LT, GT, EL], f32, name=f"o_big{tiles[0] // GT}"
                    )
                ps = psum_pool.tile([LT, q, PSW], f32, tag="ps", name="ps")
                for i, t in enumerate(tiles):
                    for k in range(K2):
                        nc.tensor.matmul(
                            ps[:, i, 0:NW],
                            u_window(t, k),
                            w_sb[:, k * NW : (k + 1) * NW],
                            start=(k == 0),
                            stop=(k == K2 - 1),
                        )

                # fone = f block broadcast over the 16 e-blocks, in ONE
                # ACT copy straight from PSUM (bias/pad cols pre-set above).
                fone = fones[g % 4]
                f4 = fone[:, 0:q, :].rearrange("p q (e j) -> p q e j", j=NJB)
                fps = ps[:, :, NA:NW]
                fbc = bass.AP(
                    tensor=fps.tensor,
                    offset=fps.offset,
                    ap=[fps.ap[0], fps.ap[1], [0, EL], fps.ap[2]],
                )
                nc.scalar.copy(out=f4[:, :, :, 0:NJ], in_=fbc)

                # fused multiply+cumsum straight from PSUM (fp32 A, bf16 f)
                cum = cums[g % 4]
                cview = cum[:, 1 : 1 + q * NA].rearrange(
                    "p (q n) -> p q n", n=NA
                )
                nc.vector._custom_dve(
                    seg_op,
                    out=cview,
                    in0=ps[:, :, 0:NA],
                    in1=fone[:, 0:q, :],
                )
                # segment sums = boundary differences of the global cumsum:
                # out[q,e] = cum[q*NA + e*NJB + NJB] - cum[q*NA + e*NJB]
                # (both views pick col j=0 of each 26-block; cum[0] is the
                # memset seed).
                cur = cum[:, NJB : NJB + q * NA].rearrange(
                    "p (q e j) -> p q e j", e=EL, j=NJB
                )[:, :, :, 0:1]
                prev = cum[:, 0 : q * NA].rearrange(
                    "p (q e j) -> p q e j", e=EL, j=NJB
                )[:, :, :, 0:1]
                o_big = o_bigs[tiles[0] // GT]
                s0 = tiles[0] % GT
                nc.vector.tensor_tensor(
                    out=o_big[:, s0 : s0 + q, :],
                    in0=cur,
                    in1=prev,
                    op=mybir.AluOpType.subtract,
                )
                og = tiles[0] // GT
                if tiles[-1] == 7:
                    nc.sync.dma_start(out=o_dram[og], in_=o_big[:])
                elif tiles[-1] == 13:
                    nc.sync.dma_start(
                        out=o_dram[og][:, 0 : 6 * EL], in_=o_big[:, 0:6, :]
                    )
                elif tiles[-1] == 15:
                    nc.sync.dma_start(
                        out=o_dram[og][:, 6 * EL : GT * EL],
                        in_=o_big[:, 6:GT, :],
                    )

    nc.compile()
    return nc


def _prep_inputs(u, proj, conv_w, conv_b):
    """Host-side layout prep: reshuffle + bf16 rounding only."""
    u_padt = np.zeros((D, L + 2), BF16)
    u_padt[:, 1 : L + 1] = np.ascontiguousarray(u[0].T).astype(BF16)

    in_maps = []
    for c in range(NCORES):
        e0 = c * EL
        w_aug = np.zeros((K2, D, NW), np.float32)
        # conv weights: m = d*K2 + k2 (in_channel-major, tap-minor)
        cw = conv_w[e0 : e0 + EL].reshape(EL, K1, F, D, K2)
        wmain = cw.transpose(4, 3, 0, 1, 2).reshape(K2, D, EL, NJ)
        wa = w_aug[:, :, :NA].reshape(K2, D, EL, NJB)
        wa[:, :, :, :NJ] = wmain
        # bias at j = 24 of each 26-wide block (multiplied by the 1.0 slot)
        cb = conv_b[e0 : e0 + EL, 0, :, 0].reshape(EL, D, K2)
        wa[:, :, :, NJ] = cb.transpose(2, 1, 0)
        # proj columns: only in the k2 == k1 matmul
        for k in range(K2):
            w_aug[k, :, NA + k * F : NA + (k + 1) * F] = proj
        w_flat = w_aug.transpose(1, 0, 2).reshape(D, K2 * NW).astype(BF16)
        in_maps.append(
            {"u_padt": u_padt, "w_aug": np.ascontiguousarray(w_flat)}
        )
    return in_maps


_PROGRAM_CACHE = {}


def kernel(
    u,
    kernel_params_feat_proj,
    kernel_params_conv_weights,
    kernel_params_conv_bias,
):
    from concourse.bass_utils import run_bass_kernel_spmd

    u = np.asarray(u, np.float32)
    proj = np.asarray(kernel_params_feat_proj, np.float32)
    conv_w = np.asarray(kernel_params_conv_weights, np.float32)
    conv_b = np.asarray(kernel_params_conv_bias, np.float32)

    if "nc" not in _PROGRAM_CACHE:
        _PROGRAM_CACHE["nc"] = _build_program()
    nc = _PROGRAM_CACHE["nc"]

    in_maps = _prep_inputs(u, proj, conv_w, conv_b)
    res = run_bass_kernel_spmd(nc, in_maps, list(range(NCORES)))

    out = np.empty((B, L, E), np.float32)
    for c in range(NCORES):
        # o_dram [NG, 128, GT, EL] with l = (g*GT + t)*128 + l_sub
        arr = res.results[c]["out"].reshape(NG, LT, GT, EL)
        arr = arr.transpose(0, 2, 1, 3).reshape(L, EL)
        out[0, :, c * EL : (c + 1) * EL] = arr
    return out


# revision 4
# speedup vs baseline: 1.0227x; 1.0227x over previous
"""DynamicConv Trainium2 kernel.

Math (B=1, L=2048, D=128, E=128, F=8, K1=K2=3, M=K2*D=384):
  f   = u @ proj                                   [L, F]
  kp[l,e,m] = sum_{k1,fc} f_pad[l+k1-1,fc] * W[e,k1,fc,m] + b[e,m]
  out[l,e]  = sum_{d,k2} u_pad[l+k2-1,d] * kp[l,e,d*K2+k2]

Factorized as out[l,e] = sum_j f_tap[l,j] * A_j[l,e] + bias_t[l,e] with
A/bias/f all produced by 3 shifted bf16 matmuls per l-tile of 128 positions
accumulated in PSUM; proj columns are embedded in the rhs so f_tap falls out
of the same matmuls.  PSUM layout (26-wide blocks):
  e*26 + j  (j<24): A_j[l,e];  j=24: bias_t[l,e];  j=25: zero
  416 + j   (j<24): f_tap[l,j]
The combine runs on a CUSTOM DVE op (registered at build time through the
documented dve_ops extension point): SEG_MUL_CUMSUM computes
  cum[p, k] = sum_{k'<=k} in0[p, k'] * in1[p, k']
in one 1x pass, reading the A/bias columns STRAIGHT FROM PSUM in fp32
(no ACT bulk copy, no separate multiply).  Per-(e) segment sums are then
boundary differences of the global cumsum:
  out[l, e] = cum[l, e*26+25] - cum[l, e*26-1]     (cum[-1] := 0)
a [128, q, 16] strided tensor_tensor subtract.  in1 is "fone" =
f_tap replicated over the 16 e-blocks with 1.0 at j=24 (bias) and 0.0 at
j=25, materialized per group by ONE broadcast ACT copy straight from the
PSUM f block (GPSIMD does no streaming work at all - its SBUF port is
shared with the DVE and any concurrent GPSIMD traffic poisons DVE ops).
Per pair-group engine cost: ACT ~0.95us, DVE ~1.25us -- everything fits
under the ~10us matmul stream, and PSUM is freed by the scan itself.

The PE warm-up streams the framework's bf16 const tile via stride-0 APs
(no memset dependency) and is trimmed to 5 matmuls; the first u chunk is
split in half so the first tile's inputs land ~1.5us earlier.  Input DMAs:
sync(w0,u3), scalar(u0a,u0b,u1), gpsimd(w1,w2,u2).  Outputs are batched
8 l-tiles per DMA; the host un-permutes.

Measured ~27.0-27.4us per core on hardware (the empty-kernel launch
floor - preamble barriers + the walrus epilogue's full semaphore wipe -
is ~13us of that); rel err ~3.9e-3 vs the fp32 reference.

E is sharded 8 ways (16 channels/core); u is replicated.
"""

import numpy as np
import ml_dtypes

BF16 = ml_dtypes.bfloat16

B, L, D = 1, 2048, 128
E, F = 128, 8
K1, K2 = 3, 3
M = K2 * D
NCORES = 8
EL = E // NCORES          # 16 output channels per core
NJ = K1 * F               # 24 (k1, fc) pairs
NJB = NJ + 2              # 26-wide blocks: A(24) + bias + zero
NA = EL * NJB             # 416 A/bias columns
NW = NA + NJ              # 440 matmul columns (f block is 24 wide)
LT = 128                  # l-tile size
NT = L // LT              # 16 l-tiles
GT = 8                    # l-tiles per output DMA group
NG = NT // GT             # output groups
UC = 4                    # l-tiles per u chunk
UCOLS = UC * LT + 2       # 514
UH = 2 * LT + 2           # 258: first chunk is split for an earlier start
NU = NT // UC             # 4 u chunks
PSW = 512                 # psum columns per sub-tile (bank-aligned)
NWARM = 7                 # PE clock-ramp matmuls before the real stream
WARMC = 512               # warm-up matmul column count
QT = 2                    # max l-tiles per group
# pairs, with the last two tiles as singles: the drain chain after the
# final matmul is fone(0.55)+scan(0.56)+diff(0.1) instead of ~2.2us.
GROUPS = [[0, 1], [2, 3], [4, 5], [6, 7], [8, 9], [10, 11], [12, 13],
          [14], [15]]

_OP_NAME = "SEG_MUL_CUMSUM_ANT"


def _ensure_custom_op():
    """Register the fused multiply+cumsum DVE op via the documented
    dve_ops extension point (idempotent)."""
    import concourse.dve_ops as dve_ops

    for op in dve_ops.OPS:
        if op.name == _OP_NAME:
            return op
    from concourse.dve_spec import AluOp, Spec, Src0, Src1, lower, scan
    from concourse.dve_spec import _has_src1
    from concourse.dve_uop import DveOpSpec

    def _ref(in0, in1, s0, s1, imm2):
        p, rest = in0.shape[0], int(np.prod(in0.shape[1:]))
        prod = (in0.astype(np.float32) * in1.astype(np.float32)).reshape(p, rest)
        return np.cumsum(prod, axis=1).reshape(in0.shape)

    spec = Spec(body=scan(AluOp.ADD, Src0 * Src1), reference=_ref)
    row = 1 + len(dve_ops.OPS)
    assert row < 0x20, "custom-DVE row field overflow"
    shas = {}
    for ver in ("v3", "v4"):
        u = lower(spec, ver=ver)
        shas[ver] = DveOpSpec(
            name=_OP_NAME, opcode=row, uops=u, rd1_en=_has_src1(spec)
        ).sha(ver)
    op = dve_ops.DveOp(_OP_NAME, spec, subdim=False, uops_sha=shas)
    dve_ops.OPS.append(op)
    dve_ops.CUSTOM_DVE_SPECS[op.name] = op.spec
    dve_ops._SUB_OPCODE_FOR_NAME[op.name] = row
    return op


def _build_program():
    import concourse.bass as bass
    import concourse.bacc as bacc
    import concourse.tile as tile
    from concourse import mybir

    # Fall back to a stock mult+reduce combine (~2.5us slower) if the
    # custom-op registration ever fails in the target environment.
    try:
        seg_op = _ensure_custom_op()
    except Exception:
        seg_op = None

    f32 = mybir.dt.float32
    bf16 = mybir.dt.bfloat16
    nc = bacc.Bacc("TRN2", target_bir_lowering=False, debug=False)

    u_dram = nc.dram_tensor("u_padt", [D, L + 2], bf16, kind="ExternalInput")
    w_dram = nc.dram_tensor("w_aug", [D, K2 * NW], bf16, kind="ExternalInput")
    o_dram = nc.dram_tensor("out", [NG, D, GT * EL], f32, kind="ExternalOutput")
    # keep-alive sink for the PE warm-up matmuls (ignored by the host)
    warm_dram = nc.dram_tensor("warm", [1, 1], bf16, kind="ExternalOutput")

    with tile.TileContext(nc) as tc:
        import contextlib

        with contextlib.ExitStack() as ctx:
            const_pool = ctx.enter_context(tc.tile_pool(name="const", bufs=1))
            psum_pool = ctx.enter_context(
                tc.tile_pool(name="psum", bufs=4, space="PSUM")
            )
            fpool = ctx.enter_context(tc.tile_pool(name="ftile", bufs=4))
            fonep = ctx.enter_context(tc.tile_pool(name="fone", bufs=4))
            cump = ctx.enter_context(tc.tile_pool(name="cum", bufs=4))
            outp = ctx.enter_context(tc.tile_pool(name="outt", bufs=2))

            # u chunks: the first 4 tiles use two 2-tile chunks (earlier
            # start + precise DMA deps); the rest use 4-tile chunks.
            u_sbs = []
            for g in range(NU):
                u_g = const_pool.tile([D, UCOLS], bf16, tag=f"u{g}", name=f"u{g}")
                u_sbs.append(u_g)
            ua = const_pool.tile([D, UH], bf16, name="ua")
            ub = const_pool.tile([D, UH], bf16, name="ub")
            w_sb = const_pool.tile([D, K2 * NW], bf16)

            def u_window(t, k):
                # [128 x 128] lhs window for tile t, shift k
                if t < 2:
                    return ua[:, t * LT + k : t * LT + k + LT]
                if t < 4:
                    return ub[:, (t - 2) * LT + k : (t - 2) * LT + k + LT]
                return u_sbs[t // UC][:, (t % UC) * LT + k : (t % UC) * LT + k + LT]

            def dma_u(g, eng):
                eng.dma_start(
                    out=u_sbs[g][:],
                    in_=u_dram[:, g * UC * LT : g * UC * LT + UCOLS],
                )

            def dma_w(k, eng):
                eng.dma_start(
                    out=w_sb[:, k * NW : (k + 1) * NW],
                    in_=w_dram[:, k * NW : (k + 1) * NW],
                )

            # need-order: w0/ua/w1 gate tile 0; the first u chunk is split
            # so the stream can start ~1.5us earlier; sync frees up early
            # for the output DMAs.
            dma_w(0, nc.sync)
            nc.scalar.dma_start(out=ua[:], in_=u_dram[:, 0:UH])
            dma_w(1, nc.gpsimd)
            nc.scalar.dma_start(out=ub[:], in_=u_dram[:, 2 * LT : 2 * LT + UH])
            dma_w(2, nc.sync)
            dma_u(1, nc.scalar)
            dma_u(2, nc.gpsimd)
            dma_u(3, nc.sync)

            # PE warm-up on the framework's pre-initialized bf16 const tile
            # via stride-0 APs: no memset dependency, so the DVFS ramp starts
            # the moment the PE enters the body.
            one_ap = nc.const_aps.aps[(bf16, 1.0)]
            warm_in0 = bass.AP(
                tensor=one_ap.tensor, offset=one_ap.offset,
                ap=[one_ap.ap[0], [0, LT]],
            )
            warm_in1 = bass.AP(
                tensor=one_ap.tensor, offset=one_ap.offset,
                ap=[one_ap.ap[0], [0, WARMC]],
            )
            warm_ps = psum_pool.tile([LT, QT, PSW], f32, tag="ps", name="warm_ps")
            for i in range(NWARM):
                nc.tensor.matmul(
                    warm_ps[:, 0, 0:WARMC],
                    warm_in0,
                    warm_in1,
                    start=(i == 0),
                    stop=(i == NWARM - 1),
                )
            warm_1 = bass.AP(
                tensor=one_ap.tensor, offset=one_ap.offset,
                ap=[[one_ap.ap[0][0], 1], [1, 1]],
            )
            nc.sync.dma_start(out=warm_dram[:], in_=warm_1)

            # rotating buffers: fone gets its 1.0 (bias) / 0.0 (pad) block
            # cols once; cum gets its seed column (global cumsum "-1" = 0).
            fones, cums = [], []
            for b in range(4):
                fone = fonep.tile([LT, QT, NA], bf16, tag="fone", name=f"fone{b}")
                f4 = fone[:].rearrange("p q (e j) -> p q e j", j=NJB)
                nc.gpsimd.memset(f4[:, :, :, NJ : NJ + 1], 1.0)
                nc.gpsimd.memset(f4[:, :, :, NJ + 1 : NJB], 0.0)
                fones.append(fone)
                # width 1 + QT*NA + NJB: the trailing NJB cols are slack so
                # the boundary-view slices stay in bounds (never read).
                cum = cump.tile(
                    [LT, 1 + QT * NA + NJB], f32, tag="cum", name=f"cum{b}"
                )
                nc.gpsimd.memset(cum[:, 0:1], 0.0)
                cums.append(cum)


            o_bigs = {}

            for g, tiles in enumerate(GROUPS):
                q = len(tiles)
                if tiles[0] % GT == 0:
                    o_bigs[tiles[0] // GT] = outp.tile(
                        [LT, GT, EL], f32, name=f"o_big{tiles[0] // GT}"
                    )
                ps = psum_pool.tile([LT, q, PSW], f32, tag="ps", name="ps")
                for i, t in enumerate(tiles):
                    for k in range(K2):
                        nc.tensor.matmul(
                            ps[:, i, 0:NW],
                            u_window(t, k),
                            w_sb[:, k * NW : (k + 1) * NW],
                            start=(k == 0),
                            stop=(k == K2 - 1),
                        )

                # fone = f block broadcast over the 16 e-blocks, in ONE
                # ACT copy straight from PSUM (bias/pad cols pre-set above).
                fone = fones[g % 4]
                f4 = fone[:, 0:q, :].rearrange("p q (e j) -> p q e j", j=NJB)
                fps = ps[:, :, NA:NW]
                fbc = bass.AP(
                    tensor=fps.tensor,
                    offset=fps.offset,
                    ap=[fps.ap[0], fps.ap[1], [0, EL], fps.ap[2]],
                )
                nc.scalar.copy(out=f4[:, :, :, 0:NJ], in_=fbc)

                o_big = o_bigs[tiles[0] // GT]
                s0 = tiles[0] % GT
                if seg_op is not None:
                    # fused multiply+cumsum straight from PSUM (fp32 A,
                    # bf16 f)
                    cum = cums[g % 4]
                    cview = cum[:, 1 : 1 + q * NA].rearrange(
                        "p (q n) -> p q n", n=NA
                    )
                    nc.vector._custom_dve(
                        seg_op,
                        out=cview,
                        in0=ps[:, :, 0:NA],
                        in1=fone[:, 0:q, :],
                    )
                    # segment sums = boundary differences of the global
                    # cumsum: out[q,e] = cum[q*NA+e*NJB+NJB] -
                    # cum[q*NA+e*NJB] (both views pick col j=0 of each
                    # 26-block; cum[0] is the memset seed).
                    cur = cum[:, NJB : NJB + q * NA].rearrange(
                        "p (q e j) -> p q e j", e=EL, j=NJB
                    )[:, :, :, 0:1]
                    prev = cum[:, 0 : q * NA].rearrange(
                        "p (q e j) -> p q e j", e=EL, j=NJB
                    )[:, :, :, 0:1]
                    nc.vector.tensor_tensor(
                        out=o_big[:, s0 : s0 + q, :],
                        in0=cur,
                        in1=prev,
                        op=mybir.AluOpType.subtract,
                    )
                else:
                    # stock fallback: TT multiply from PSUM + 1x reduce
                    cum = cums[g % 4]
                    prod = cum[:, 0 : q * NA].rearrange(
                        "p (q n) -> p q n", n=NA
                    )
                    nc.vector.tensor_tensor(
                        out=prod,
                        in0=ps[:, :, 0:NA],
                        in1=fone[:, 0:q, :],
                        op=mybir.AluOpType.mult,
                    )
                    nc.vector.reduce_sum(
                        out=o_big[:, s0 : s0 + q, :],
                        in_=prod.rearrange(
                            "p q (e j) -> p q e j", j=NJB
                        )[:, :, :, 0 : NJ + 1],
                        axis=mybir.AxisListType.X,
                    )
                og = tiles[0] // GT
                if tiles[-1] == 7:
                    nc.sync.dma_start(out=o_dram[og], in_=o_big[:])
                elif tiles[-1] == 13:
                    nc.sync.dma_start(
                        out=o_dram[og][:, 0 : 6 * EL], in_=o_big[:, 0:6, :]
                    )
                elif tiles[-1] == 15:
                    nc.sync.dma_start(
                        out=o_dram[og][:, 6 * EL : GT * EL],
                        in_=o_big[:, 6:GT, :],
                    )

    nc.compile()
    return nc


def _prep_inputs(u, proj, conv_w, conv_b):
    """Host-side layout prep: reshuffle + bf16 rounding only."""
    u_padt = np.zeros((D, L + 2), BF16)
    u_padt[:, 1 : L + 1] = np.ascontiguousarray(u[0].T).astype(BF16)

    in_maps = []
    for c in range(NCORES):
        e0 = c * EL
        w_aug = np.zeros((K2, D, NW), np.float32)
        # conv weights: m = d*K2 + k2 (in_channel-major, tap-minor)
        cw = conv_w[e0 : e0 + EL].reshape(EL, K1, F, D, K2)
        wmain = cw.transpose(4, 3, 0, 1, 2).reshape(K2, D, EL, NJ)
        wa = w_aug[:, :, :NA].reshape(K2, D, EL, NJB)
        wa[:, :, :, :NJ] = wmain
        # bias at j = 24 of each 26-wide block (multiplied by the 1.0 slot)
        cb = conv_b[e0 : e0 + EL, 0, :, 0].reshape(EL, D, K2)
        wa[:, :, :, NJ] = cb.transpose(2, 1, 0)
        # proj columns: only in the k2 == k1 matmul
        for k in range(K2):
            w_aug[k, :, NA + k * F : NA + (k + 1) * F] = proj
        w_flat = w_aug.transpose(1, 0, 2).reshape(D, K2 * NW).astype(BF16)
        in_maps.append(
            {"u_padt": u_padt, "w_aug": np.ascontiguousarray(w_flat)}
        )
    return in_maps


_PROGRAM_CACHE = {}


def kernel(
    u,
    kernel_params_feat_proj,
    kernel_params_conv_weights,
    kernel_params_conv_bias,
):
    from concourse.bass_utils import run_bass_kernel_spmd

    u = np.asarray(u, np.float32)
    proj = np.asarray(kernel_params_feat_proj, np.float32)
    conv_w = np.asarray(kernel_params_conv_weights, np.float32)
    conv_b = np.asarray(kernel_params_conv_bias, np.float32)

    if "nc" not in _PROGRAM_CACHE:
        _PROGRAM_CACHE["nc"] = _build_program()
    nc = _PROGRAM_CACHE["nc"]

    in_maps = _prep_inputs(u, proj, conv_w, conv_b)
    res = run_bass_kernel_spmd(nc, in_maps, list(range(NCORES)))

    out = np.empty((B, L, E), np.float32)
    for c in range(NCORES):
        # o_dram [NG, 128, GT, EL] with l = (g*GT + t)*128 + l_sub
        arr = res.results[c]["out"].reshape(NG, LT, GT, EL)
        arr = arr.transpose(0, 2, 1, 3).reshape(L, EL)
        out[0, :, c * EL : (c + 1) * EL] = arr
    return out


# revision 8
# speedup vs baseline: 1.0584x; 1.0349x over previous
"""DynamicConv Trainium2 kernel.

Math (B=1, L=2048, D=128, E=128, F=8, K1=K2=3, M=K2*D=384):
  f   = u @ proj                                   [L, F]
  kp[l,e,m] = sum_{k1,fc} f_pad[l+k1-1,fc] * W[e,k1,fc,m] + b[e,m]
  out[l,e]  = sum_{d,k2} u_pad[l+k2-1,d] * kp[l,e,d*K2+k2]

Factorized as out[l,e] = sum_j f_tap[l,j] * A_j[l,e] + bias_t[l,e] with
A/bias/f all produced by 3 shifted bf16 matmuls per l-tile of 128 positions
accumulated in PSUM; proj columns are embedded in the rhs so f_tap falls out
of the same matmuls.  PSUM layout (25-wide blocks):
  e*25 + j  (j<24): A_j[l,e];  j=24: bias_t[l,e]
  400 + j   (j<24): f_tap[l,j]
The combine runs on a CUSTOM DVE op (registered at build time through the
documented dve_ops extension point): SEG_MUL_CUMSUM computes
  cum[p, k] = sum_{k'<=k} in0[p, k'] * in1[p, k']
in one 1x pass, reading the A/bias columns STRAIGHT FROM PSUM in fp32
(no ACT bulk copy, no separate multiply).  Per-(e) segment sums are then
boundary differences of the global cumsum:
  out[l, e] = cum[l, e*25+24] - cum[l, e*25-1]     (cum[-1] := 0)
a [128, q, 16] strided tensor_tensor subtract.  in1 is "fone" =
f_tap replicated over the 16 e-blocks with 1.0 in the bias slot (j=24),
materialized per group by ONE broadcast ACT copy straight from the
PSUM f block (GPSIMD does no streaming work at all - its SBUF port is
shared with the DVE and any concurrent GPSIMD traffic poisons DVE ops).
Per pair-group engine cost: ACT ~0.95us, DVE ~1.25us -- everything fits
under the ~10us matmul stream, and PSUM is freed by the scan itself.

The PE warm-up streams the framework's bf16 const tile via stride-0 APs
(no memset dependency) for 7 matmuls; the first u chunk is
split in half so the first tile's inputs land ~1.5us earlier.  Input DMAs:
sync(w0,u3), scalar(u0a,u0b,u1), gpsimd(w1,w2,u2).  Outputs are batched
8 l-tiles per DMA; the host un-permutes.

Measured ~26.6-28us per core on hardware (the empty-kernel launch floor -
preamble barriers + the walrus epilogue's full semaphore wipe - is ~13us
of that); rel err ~3.9e-3 vs the fp32 reference.

E is sharded 8 ways (16 channels/core); u is replicated.
"""

import numpy as np
import ml_dtypes

BF16 = ml_dtypes.bfloat16

B, L, D = 1, 2048, 128
E, F = 128, 8
K1, K2 = 3, 3
M = K2 * D
NCORES = 8
EL = E // NCORES          # 16 output channels per core
NJ = K1 * F               # 24 (k1, fc) pairs
NJB = NJ + 1              # 25-wide blocks: A(24) + bias (the fused scan
                          # runs at 1x, so no even-alignment pad is needed)
NA = EL * NJB             # 400 A/bias columns
NW = NA + NJ              # 424 matmul columns (f block is 24 wide)
LT = 128                  # l-tile size
NT = L // LT              # 16 l-tiles
GT = 8                    # l-tiles per output DMA group
NG = NT // GT             # output groups
UC = 4                    # l-tiles per u chunk
UCOLS = UC * LT + 2       # 514
UH = 2 * LT + 2           # 258: first chunk is split for an earlier start
NU = NT // UC             # 4 u chunks
PSW = 512                 # psum columns per sub-tile (bank-aligned)
NWARM = 7                 # PE clock-ramp matmuls before the real stream
WARMC = 512               # warm-up matmul column count
QT = 2                    # max l-tiles per group
# pairs, with the last two tiles as singles: the drain chain after the
# final matmul is fone(0.55)+scan(0.56)+diff(0.1) instead of ~2.2us.
GROUPS = [[0, 1], [2, 3], [4, 5], [6, 7], [8, 9], [10, 11], [12, 13],
          [14], [15]]

_OP_NAME = "SEG_MUL_CUMSUM_ANT"


def _ensure_custom_op():
    """Register the fused multiply+cumsum DVE op via the documented
    dve_ops extension point (idempotent)."""
    import concourse.dve_ops as dve_ops

    for op in dve_ops.OPS:
        if op.name == _OP_NAME:
            return op
    from concourse.dve_spec import AluOp, Spec, Src0, Src1, lower, scan
    from concourse.dve_spec import _has_src1
    from concourse.dve_uop import DveOpSpec

    def _ref(in0, in1, s0, s1, imm2):
        p, rest = in0.shape[0], int(np.prod(in0.shape[1:]))
        prod = (in0.astype(np.float32) * in1.astype(np.float32)).reshape(p, rest)
        return np.cumsum(prod, axis=1).reshape(in0.shape)

    spec = Spec(body=scan(AluOp.ADD, Src0 * Src1), reference=_ref)
    row = 1 + len(dve_ops.OPS)
    assert row < 0x20, "custom-DVE row field overflow"
    shas = {}
    for ver in ("v3", "v4"):
        u = lower(spec, ver=ver)
        shas[ver] = DveOpSpec(
            name=_OP_NAME, opcode=row, uops=u, rd1_en=_has_src1(spec)
        ).sha(ver)
    op = dve_ops.DveOp(_OP_NAME, spec, subdim=False, uops_sha=shas)
    dve_ops.OPS.append(op)
    dve_ops.CUSTOM_DVE_SPECS[op.name] = op.spec
    dve_ops._SUB_OPCODE_FOR_NAME[op.name] = row
    return op


def _build_program():
    import concourse.bass as bass
    import concourse.bacc as bacc
    import concourse.tile as tile
    from concourse import mybir

    # Fall back to a stock mult+reduce combine (~2.5us slower) if the
    # custom-op registration ever fails in the target environment.
    try:
        seg_op = _ensure_custom_op()
    except Exception:
        seg_op = None

    f32 = mybir.dt.float32
    bf16 = mybir.dt.bfloat16
    nc = bacc.Bacc("TRN2", target_bir_lowering=False, debug=False)

    u_dram = nc.dram_tensor("u_padt", [D, L + 2], bf16, kind="ExternalInput")
    w_dram = nc.dram_tensor("w_aug", [D, K2 * NW], bf16, kind="ExternalInput")
    o_dram = nc.dram_tensor("out", [NG, D, GT * EL], f32, kind="ExternalOutput")
    # keep-alive sink for the PE warm-up matmuls (ignored by the host)
    warm_dram = nc.dram_tensor("warm", [1, 1], bf16, kind="ExternalOutput")

    with tile.TileContext(nc) as tc:
        import contextlib

        with contextlib.ExitStack() as ctx:
            const_pool = ctx.enter_context(tc.tile_pool(name="const", bufs=1))
            psum_pool = ctx.enter_context(
                tc.tile_pool(name="psum", bufs=4, space="PSUM")
            )
            fpool = ctx.enter_context(tc.tile_pool(name="ftile", bufs=4))
            fonep = ctx.enter_context(tc.tile_pool(name="fone", bufs=4))
            cump = ctx.enter_context(tc.tile_pool(name="cum", bufs=4))
            outp = ctx.enter_context(tc.tile_pool(name="outt", bufs=2))

            # u chunks: the first 4 tiles use two 2-tile chunks (earlier
            # start + precise DMA deps); the rest use 4-tile chunks.
            u_sbs = []
            for g in range(NU):
                u_g = const_pool.tile([D, UCOLS], bf16, tag=f"u{g}", name=f"u{g}")
                u_sbs.append(u_g)
            ua = const_pool.tile([D, UH], bf16, name="ua")
            ub = const_pool.tile([D, UH], bf16, name="ub")
            w_sb = const_pool.tile([D, K2 * NW], bf16)

            def u_window(t, k):
                # [128 x 128] lhs window for tile t, shift k
                if t < 2:
                    return ua[:, t * LT + k : t * LT + k + LT]
                if t < 4:
                    return ub[:, (t - 2) * LT + k : (t - 2) * LT + k + LT]
                return u_sbs[t // UC][:, (t % UC) * LT + k : (t % UC) * LT + k + LT]

            def dma_u(g, eng):
                eng.dma_start(
                    out=u_sbs[g][:],
                    in_=u_dram[:, g * UC * LT : g * UC * LT + UCOLS],
                )

            def dma_w(k, eng):
                eng.dma_start(
                    out=w_sb[:, k * NW : (k + 1) * NW],
                    in_=w_dram[:, k * NW : (k + 1) * NW],
                )

            # need-order: w0/ua/w1 gate tile 0; the first u chunk is split
            # so the stream can start ~1.5us earlier; sync frees up early
            # for the output DMAs.
            dma_w(0, nc.sync)
            nc.scalar.dma_start(out=ua[:], in_=u_dram[:, 0:UH])
            dma_w(1, nc.gpsimd)
            nc.scalar.dma_start(out=ub[:], in_=u_dram[:, 2 * LT : 2 * LT + UH])
            dma_w(2, nc.sync)
            dma_u(1, nc.scalar)
            dma_u(2, nc.gpsimd)
            dma_u(3, nc.sync)

            # PE warm-up on the framework's pre-initialized bf16 const tile
            # via stride-0 APs: no memset dependency, so the DVFS ramp starts
            # the moment the PE enters the body.
            one_ap = nc.const_aps.aps[(bf16, 1.0)]
            warm_in0 = bass.AP(
                tensor=one_ap.tensor, offset=one_ap.offset,
                ap=[one_ap.ap[0], [0, LT]],
            )
            warm_in1 = bass.AP(
                tensor=one_ap.tensor, offset=one_ap.offset,
                ap=[one_ap.ap[0], [0, WARMC]],
            )
            warm_ps = psum_pool.tile([LT, QT, PSW], f32, tag="ps", name="warm_ps")
            for i in range(NWARM):
                nc.tensor.matmul(
                    warm_ps[:, 0, 0:WARMC],
                    warm_in0,
                    warm_in1,
                    start=(i == 0),
                    stop=(i == NWARM - 1),
                )
            warm_1 = bass.AP(
                tensor=one_ap.tensor, offset=one_ap.offset,
                ap=[[one_ap.ap[0][0], 1], [1, 1]],
            )
            nc.sync.dma_start(out=warm_dram[:], in_=warm_1)

            # rotating buffers: fone gets its 1.0 (bias) / 0.0 (pad) block
            # cols once; cum gets its seed column (global cumsum "-1" = 0).
            fones, cums = [], []
            for b in range(4):
                fone = fonep.tile([LT, QT, NA], bf16, tag="fone", name=f"fone{b}")
                f4 = fone[:].rearrange("p q (e j) -> p q e j", j=NJB)
                nc.gpsimd.memset(f4[:, :, :, NJ : NJB], 1.0)
                fones.append(fone)
                # width 1 + QT*NA + NJB: the trailing NJB cols are slack so
                # the boundary-view slices stay in bounds (never read).
                cum = cump.tile(
                    [LT, 1 + QT * NA + NJB], f32, tag="cum", name=f"cum{b}"
                )
                nc.gpsimd.memset(cum[:, 0:1], 0.0)
                cums.append(cum)


            o_bigs = {}

            for g, tiles in enumerate(GROUPS):
                q = len(tiles)
                if tiles[0] % GT == 0:
                    o_bigs[tiles[0] // GT] = outp.tile(
                        [LT, GT, EL], f32, name=f"o_big{tiles[0] // GT}"
                    )
                ps = psum_pool.tile([LT, q, PSW], f32, tag="ps", name="ps")
                for i, t in enumerate(tiles):
                    for k in range(K2):
                        nc.tensor.matmul(
                            ps[:, i, 0:NW],
                            u_window(t, k),
                            w_sb[:, k * NW : (k + 1) * NW],
                            start=(k == 0),
                            stop=(k == K2 - 1),
                        )

                # fone = f block broadcast over the 16 e-blocks, in ONE
                # ACT copy straight from PSUM (bias/pad cols pre-set above).
                fone = fones[g % 4]
                f4 = fone[:, 0:q, :].rearrange("p q (e j) -> p q e j", j=NJB)
                fps = ps[:, :, NA:NW]
                fbc = bass.AP(
                    tensor=fps.tensor,
                    offset=fps.offset,
                    ap=[fps.ap[0], fps.ap[1], [0, EL], fps.ap[2]],
                )
                nc.scalar.copy(out=f4[:, :, :, 0:NJ], in_=fbc)

                o_big = o_bigs[tiles[0] // GT]
                s0 = tiles[0] % GT
                cum = cums[g % 4]
                if seg_op is not None:
                    # fused multiply+cumsum straight from PSUM (fp32 A,
                    # bf16 f)
                    cview = cum[:, 1 : 1 + q * NA].rearrange(
                        "p (q n) -> p q n", n=NA
                    )
                    nc.vector._custom_dve(
                        seg_op,
                        out=cview,
                        in0=ps[:, :, 0:NA],
                        in1=fone[:, 0:q, :],
                    )
                    # segment sums = boundary differences of the global
                    # cumsum: out[q,e] = cum[q*NA+e*NJB+NJB] -
                    # cum[q*NA+e*NJB] (both views pick col j=0 of each
                    # 25-block; cum[0] is the memset seed).
                    cur = cum[:, NJB : NJB + q * NA].rearrange(
                        "p (q e j) -> p q e j", e=EL, j=NJB
                    )[:, :, :, 0:1]
                    prev = cum[:, 0 : q * NA].rearrange(
                        "p (q e j) -> p q e j", e=EL, j=NJB
                    )[:, :, :, 0:1]
                    nc.vector.tensor_tensor(
                        out=o_big[:, s0 : s0 + q, :],
                        in0=cur,
                        in1=prev,
                        op=mybir.AluOpType.subtract,
                    )
                else:
                    # stock fallback: TT multiply from PSUM + 1x reduce
                    prod = cum[:, 0 : q * NA].rearrange(
                        "p (q n) -> p q n", n=NA
                    )
                    nc.vector.tensor_tensor(
                        out=prod,
                        in0=ps[:, :, 0:NA],
                        in1=fone[:, 0:q, :],
                        op=mybir.AluOpType.mult,
                    )
                    nc.vector.reduce_sum(
                        out=o_big[:, s0 : s0 + q, :],
                        in_=prod.rearrange(
                            "p q (e j) -> p q e j", j=NJB
                        ),
                        axis=mybir.AxisListType.X,
                    )
                og = tiles[0] // GT
                if tiles[-1] == 7:
                    nc.sync.dma_start(out=o_dram[og], in_=o_big[:])
                elif tiles[-1] == 13:
                    nc.sync.dma_start(
                        out=o_dram[og][:, 0 : 6 * EL], in_=o_big[:, 0:6, :]
                    )
                elif tiles[-1] == 15:
                    nc.sync.dma_start(
                        out=o_dram[og][:, 6 * EL : GT * EL],
                        in_=o_big[:, 6:GT, :],
                    )

    nc.compile()
    return nc


def _prep_inputs(u, proj, conv_w, conv_b):
    """Host-side layout prep: reshuffle + bf16 rounding only."""
    u_padt = np.zeros((D, L + 2), BF16)
    u_padt[:, 1 : L + 1] = np.ascontiguousarray(u[0].T).astype(BF16)

    in_maps = []
    for c in range(NCORES):
        e0 = c * EL
        w_aug = np.zeros((K2, D, NW), np.float32)
        # conv weights: m = d*K2 + k2 (in_channel-major, tap-minor)
        cw = conv_w[e0 : e0 + EL].reshape(EL, K1, F, D, K2)
        wmain = cw.transpose(4, 3, 0, 1, 2).reshape(K2, D, EL, NJ)
        wa = w_aug[:, :, :NA].reshape(K2, D, EL, NJB)
        wa[:, :, :, :NJ] = wmain
        # bias at j = 24 of each 26-wide block (multiplied by the 1.0 slot)
        cb = conv_b[e0 : e0 + EL, 0, :, 0].reshape(EL, D, K2)
        wa[:, :, :, NJ] = cb.transpose(2, 1, 0)
        # proj columns: only in the k2 == k1 matmul
        for k in range(K2):
            w_aug[k, :, NA + k * F : NA + (k + 1) * F] = proj
        w_flat = w_aug.transpose(1, 0, 2).reshape(D, K2 * NW).astype(BF16)
        in_maps.append(
            {"u_padt": u_padt, "w_aug": np.ascontiguousarray(w_flat)}
        )
    return in_maps


_PROGRAM_CACHE = {}


def kernel(
    u,
    kernel_params_feat_proj,
    kernel_params_conv_weights,
    kernel_params_conv_bias,
):
    from concourse.bass_utils import run_bass_kernel_spmd

    u = np.asarray(u, np.float32)
    proj = np.asarray(kernel_params_feat_proj, np.float32)
    conv_w = np.asarray(kernel_params_conv_weights, np.float32)
    conv_b = np.asarray(kernel_params_conv_bias, np.float32)

    if "nc" not in _PROGRAM_CACHE:
        _PROGRAM_CACHE["nc"] = _build_program()
    nc = _PROGRAM_CACHE["nc"]

    in_maps = _prep_inputs(u, proj, conv_w, conv_b)
    res = run_bass_kernel_spmd(nc, in_maps, list(range(NCORES)))

    out = np.empty((B, L, E), np.float32)
    for c in range(NCORES):
        # o_dram [NG, 128, GT, EL] with l = (g*GT + t)*128 + l_sub
        arr = res.results[c]["out"].reshape(NG, LT, GT, EL)
        arr = arr.transpose(0, 2, 1, 3).reshape(L, EL)
        out[0, :, c * EL : (c + 1) * EL] = arr
    return out
